# revision 17
# baseline (speedup 1.0000x reference)
"""Multi-head attention (B=4, S=2048, D=1024, H=16) on 8 Trainium2 cores.

Sharding (v5): core c -> head-pair p = c (2 heads, 128 output dims), all 4
batches.  This makes the valid_len truncation SPMD-uniform: every core runs
the same per-batch k-loop trip counts kc_b = ceil(valid_len[b]/128), so the
~50% of attention work beyond the key-padding boundary is simply never
emitted.  W_o is row-split by head-pair; each core emits a full-shape
[B, S, D] fp16 partial and the host sums the 8 partials.

Masking: the host zeroes xv columns at k >= valid_len[b] and supplies a
masked ones-column, so masked keys contribute exactly 0 to both the
attention*V accumulation and the softmax denominator.  exp then needs no
bias at all (scores at masked-but-computed boundary columns are finite).

Device layout notes:
  - matmul computes lhsT.T @ rhs with contraction on the partition dim.
  - Q/K are produced transposed ([dout, s]) so scores come out as
    scores^T [k, q]; 1/sqrt(hd) folds into the ScalarE exp op's scale.
  - V is natural [s, dout] with a (host-masked) ones column per head, so the
    attention*V matmul also emits the softmax denominators.
  - All matmul inputs bf16 (4x faster PE than fp32), fp32 PSUM accumulation.
  - PSUM budget: scp 2x2 banks + av 2 + projection 2 = 8; Q/K projection of
    batch b+1 is emitted ahead of attention of batch b so the PE fills the
    gaps of the ACT-bound attention pipeline.
"""

import contextlib

import numpy as np
import ml_dtypes

import concourse.bacc as bacc
import concourse.mybir as mybir
import concourse.tile as tile
from concourse.bass_utils import run_bass_kernel_spmd

BF16 = mybir.dt.bfloat16
F16 = mybir.dt.float16
F32 = mybir.dt.float32
AF = mybir.ActivationFunctionType

B, S, D, H, HD = 4, 2048, 1024, 16, 64
NQB = S // 512        # query blocks of 512

_cache = {}


def _qk_proj_b(nc, b, kcb, ap, stream, psum, wq_sb, wk_sb, qT_sb, kT_sb):
    """Q/K projection for one batch (both heads of the pair = 128 dims)."""
    for qb in range(NQB):
        psq = psum.tile([128, 512], F32, tag="pqk", name="psq")
        for dj in range(8):
            xqt = stream.tile([128, 512], BF16, tag="xq", name="xqt")
            nc.sync.dma_start(
                xqt[:],
                ap[f"xq{b}"][dj * 128:(dj + 1) * 128,
                             qb * 512:(qb + 1) * 512])
            nc.tensor.matmul(psq[:], wq_sb[dj][:], xqt[:],
                             start=(dj == 0), stop=(dj == 7))
        nc.vector.tensor_copy(qT_sb[b][:, qb * 512:(qb + 1) * 512], psq[:])
    nk = kcb * 128
    for kb in range((nk + 511) // 512):
        n = min(512, nk - kb * 512)
        psk = psum.tile([128, 512], F32, tag="pqk", name="psk")
        for dj in range(8):
            xkt = stream.tile([128, 512], BF16, tag="xk", name="xkt")
            nc.sync.dma_start(
                xkt[:, 0:n],
                ap[f"xk{b}"][dj * 128:(dj + 1) * 128,
                             kb * 512:kb * 512 + n])
            nc.tensor.matmul(psk[:, 0:n], wk_sb[dj][:], xkt[:, 0:n],
                             start=(dj == 0), stop=(dj == 7))
        nc.vector.tensor_copy(
            kT_sb[b][:, kb * 512:kb * 512 + n], psk[:, 0:n])


def _v_proj_b(nc, b, kcb, ap, stream, psum, wv_sb, v_sb, vm_sb):
    """V projection for one batch, natural [s, dout] + host-masked ones
    column per head.  v_sb[b][sc] is [128, 2, HD+1]."""
    xv_sb = []
    for dj in range(8):
        xvt = stream.tile([128, 2048], BF16, tag="xv", name="xvt", bufs=9)
        nc.sync.dma_start(
            xvt[:, 0:kcb * 128],
            ap[f"xv{b}"][dj * 128:(dj + 1) * 128, :])
        xv_sb.append(xvt)
    for sc in range(kcb):
        psv = psum.tile([128, 2, HD], F32, tag="pv", name="psv")
        for dj in range(8):
            nc.tensor.matmul(
                psv[:], xv_sb[dj][:, sc * 128:(sc + 1) * 128],
                wv_sb[dj][:], start=(dj == 0), stop=(dj == 7))
        nc.vector.tensor_copy(v_sb[b][sc][:, :, 0:HD], psv[:])
        nc.vector.tensor_copy(
            v_sb[b][sc][:, :, HD], vm_sb[:, b * 16 + sc, :])


def _attention_b(nc, b, kcb, psum, expool, wrk, qT_sb, kT_sb, v_sb, ctx_sb):
    """Attention for one batch (2 heads): ACT-bound pipeline over (qb, kc)."""
    for qb in range(4):
        av = psum.tile([HD + 1, 2, 512], F32, tag="av", name="av")
        for kc in range(kcb):
            scp = psum.tile([128, 2, 512], F32, tag="sc", name="scp", bufs=2)
            for h2 in range(2):
                nc.tensor.matmul(
                    scp[:, h2, :],
                    kT_sb[b][64 * h2:64 * h2 + 64, kc * 128:(kc + 1) * 128],
                    qT_sb[b][64 * h2:64 * h2 + 64, qb * 512:(qb + 1) * 512],
                    start=True, stop=True)
            ex = expool.tile([128, 2, 512], BF16, tag="ex", name="ex")
            nc.scalar.activation(ex[:], scp[:], AF.Exp, scale=0.125)
            for h2 in range(2):
                nc.tensor.matmul(
                    av[:, h2, :], v_sb[b][kc][:, h2, :], ex[:, h2, :],
                    start=(kc == 0), stop=(kc == kcb - 1))
        # Copy av to SBUF (frees the PSUM slot), then normalize off the
        # critical path: ctx[m, q] = av[m, q] / av[64, q].
        avc = wrk.tile([HD + 1, 2, 512], F32, tag="avc", name="avc")
        nc.vector.tensor_copy(avc[:, 0, :], av[:, 0, :])
        nc.scalar.activation(avc[:, 1, :], av[:, 1, :], AF.Copy)
        # Broadcast the sums row to partitions 0..63, then approx-recip there
        # (reciprocal_approx_fast breaks on 1-partition APs, and
        # partition_broadcast only reads partition 0 of its source).
        r0 = wrk.tile([1, 1024], F32, tag="r0", name="r0")
        nc.sync.dma_start(r0[:], avc[HD:HD + 1, :, :])
        bc = wrk.tile([HD, 1024], F32, tag="bc", name="bc")
        nc.gpsimd.partition_broadcast(bc[:], r0[0:1, :])
        recb = wrk.tile([HD, 1024], F32, tag="recb", name="recb")
        nc.vector.reciprocal_approx_fast(recb[:], bc[:])
        nc.vector.tensor_mul(
            ctx_sb[b][0:HD, qb * 512:(qb + 1) * 512],
            avc[0:HD, 0, :], recb[:, 0:512])
        tmp = wrk.tile([HD, 512], BF16, tag="tmpb", name="tmp")
        nc.vector.tensor_mul(tmp[:], avc[0:HD, 1, :], recb[:, 512:1024])
        nc.sync.dma_start(
            ctx_sb[b][HD:128, qb * 512:(qb + 1) * 512], tmp[:])


def _o_proj_b(nc, b, ap, psum, wrk, ctx_sb, wo_sb):
    """Output projection partial for one batch; alternates the two fill-pool
    PSUM slots (pqk/pv) so it pipelines 2-deep."""
    for sc in range(16):
        for ih in range(2):
            tg = "pqk" if (2 * sc + ih) % 2 == 0 else "pv"
            po = psum.tile([128, 512], F32, tag=tg, name="po")
            nc.tensor.matmul(
                po[:], ctx_sb[b][:, sc * 128:(sc + 1) * 128],
                wo_sb[0][:, ih * 512:(ih + 1) * 512],
                start=True, stop=True)
            ot = wrk.tile([128, 512], F16, tag="ot", name="ot", bufs=4)
            if ih == 0:
                nc.vector.tensor_copy(ot[:], po[:])
            else:
                nc.scalar.activation(ot[:], po[:], AF.Copy)
            nc.sync.dma_start(
                ap["out"][b, sc * 128:(sc + 1) * 128,
                          ih * 512:(ih + 1) * 512], ot[:])


def _emit(nc, tc, ap, kcs):
    es = contextlib.ExitStack()
    with es:
        const = es.enter_context(tc.tile_pool(name="const", bufs=1))
        resid = es.enter_context(tc.tile_pool(name="resid", bufs=1))
        stream = es.enter_context(tc.tile_pool(name="stream", bufs=3))
        expool = es.enter_context(tc.tile_pool(name="expool", bufs=3))
        wrk = es.enter_context(tc.tile_pool(name="wrk", bufs=2))

        # constants: per-dj [din-chunk, dout=128] weight tiles for the pair
        wq_sb = [const.tile([128, 128], BF16, tag=f"wq{i}", name=f"wq{i}")
                 for i in range(8)]
        wk_sb = [const.tile([128, 128], BF16, tag=f"wk{i}", name=f"wk{i}")
                 for i in range(8)]
        wv_sb = [const.tile([128, 2, HD], BF16, tag=f"wv{i}", name=f"wv{i}")
                 for i in range(8)]
        wo_sb = [const.tile([128, D], BF16, tag="wo", name="wo")]
        vm_sb = const.tile([128, 64, 2], BF16, tag="vmask", name="vmask")
        nc.sync.dma_start(vm_sb[:], ap["vones"])
        for i in range(8):
            nc.sync.dma_start(wq_sb[i][:], ap["wq"][i * 128:(i + 1) * 128, :])
            nc.sync.dma_start(wk_sb[i][:], ap["wk"][i * 128:(i + 1) * 128, :])
            nc.sync.dma_start(wv_sb[i][:],
                              ap["wv"][i * 128:(i + 1) * 128, :, :])
        nc.sync.dma_start(wo_sb[0][:], ap["wo"])

        # residents (per batch)
        qT_sb = [resid.tile([128, S], BF16, tag=f"qT{b}", name=f"qT{b}")
                 for b in range(B)]
        kT_sb = [resid.tile([128, kcs[b] * 128], BF16, tag=f"kT{b}",
                            name=f"kT{b}") for b in range(B)]
        ctx_sb = [resid.tile([128, S], BF16, tag=f"ctx{b}", name=f"ctx{b}")
                  for b in range(B)]
        v_sb = [[resid.tile([128, 2, HD + 1], BF16, tag=f"v{b}_{i}",
                            name=f"v{b}_{i}") for i in range(kcs[b])]
                for b in range(B)]

        # Fill pool (2 banks: pqk + pv) carries Q/K/V projections and the
        # O-projection; they run in the PE gaps of the ACT-bound attention.
        order = sorted(range(B), key=lambda b: -kcs[b])
        with tc.tile_pool(name="fill_psum", bufs=1, space="PSUM") as fill:
            b0 = order[0]
            _qk_proj_b(nc, b0, kcs[b0], ap, stream, fill,
                       wq_sb, wk_sb, qT_sb, kT_sb)
            _v_proj_b(nc, b0, kcs[b0], ap, stream, fill, wv_sb, v_sb, vm_sb)
            with tc.tile_pool(name="at_psum", bufs=1, space="PSUM") as at_psum:
                for i, b in enumerate(order):
                    if i + 1 < B:
                        nb = order[i + 1]
                        _qk_proj_b(nc, nb, kcs[nb], ap, stream, fill,
                                   wq_sb, wk_sb, qT_sb, kT_sb)
                        _v_proj_b(nc, nb, kcs[nb], ap, stream, fill,
                                  wv_sb, v_sb, vm_sb)
                    _attention_b(nc, b, kcs[b], at_psum, expool, wrk,
                                 qT_sb, kT_sb, v_sb, ctx_sb)
                    _o_proj_b(nc, b, ap, fill, wrk, ctx_sb, wo_sb)


def _build(kcs):
    key = ("nc", tuple(kcs))
    if key in _cache:
        return _cache[key]
    nc = bacc.Bacc("TRN2", target_bir_lowering=False, debug=False, num_devices=8)
    ap = {"wq": nc.dram_tensor("wq", [D, 128], BF16, kind="ExternalInput").ap(),
          "wk": nc.dram_tensor("wk", [D, 128], BF16, kind="ExternalInput").ap(),
          "wv": nc.dram_tensor("wv", [D, 2, HD], BF16, kind="ExternalInput").ap(),
          "wo": nc.dram_tensor("wo", [128, D], BF16, kind="ExternalInput").ap(),
          "vones": nc.dram_tensor("vones", [128, 64, 2], BF16,
                                  kind="ExternalInput").ap(),
          "out": nc.dram_tensor("out", [B, S, D], F16,
                                kind="ExternalOutput").ap()}
    for b in range(B):
        ap[f"xq{b}"] = nc.dram_tensor(f"xq{b}", [D, S], BF16,
                                      kind="ExternalInput").ap()
        ap[f"xk{b}"] = nc.dram_tensor(f"xk{b}", [D, kcs[b] * 128], BF16,
                                      kind="ExternalInput").ap()
        ap[f"xv{b}"] = nc.dram_tensor(f"xv{b}", [D, kcs[b] * 128], BF16,
                                      kind="ExternalInput").ap()
    with tile.TileContext(nc) as tc:
        _emit(nc, tc, ap, kcs)
    nc.compile()
    _cache[key] = nc
    return nc


def _in_maps(kcs, queries, keys, values, valid_len, W_q, W_k, W_v, W_o):
    bf = ml_dtypes.bfloat16
    # host-masked ones column: 1 where k < valid_len[b], else 0
    # vones[p, b*16+sc, h] = 1 if sc*128+p < valid_len[b] else 0
    kpos = np.arange(16 * 128).reshape(16, 128)
    vones = np.zeros((128, 64, 2), bf)
    for b in range(B):
        v1 = (kpos < int(valid_len[b])).astype(bf)  # [16, 128]
        vones[:, b * 16:(b + 1) * 16, :] = v1.T[:, :, None]
    maps = []
    for c in range(8):
        j0 = 128 * c
        m = {
            "wq": np.ascontiguousarray(W_q[j0:j0 + 128, :].T).astype(bf),
            "wk": np.ascontiguousarray(W_k[j0:j0 + 128, :].T).astype(bf),
            "wv": np.ascontiguousarray(
                W_v[j0:j0 + 128, :].T).astype(bf).reshape(D, 2, HD),
            "wo": np.ascontiguousarray(W_o[:, j0:j0 + 128].T).astype(bf),
            "vones": vones,
        }
        for b in range(B):
            nk = kcs[b] * 128
            xv = values[b][:nk].T.copy()      # [D, nk]
            xv[:, int(valid_len[b]):] = 0.0   # mask padding rows of v
            m[f"xq{b}"] = np.ascontiguousarray(queries[b].T).astype(bf)
            m[f"xk{b}"] = np.ascontiguousarray(keys[b][:nk].T).astype(bf)
            m[f"xv{b}"] = xv.astype(bf)
        maps.append(m)
    return maps


def kernel(queries, keys, values, valid_len, W_q, W_k, W_v, W_o, _run_kwargs=None):
    queries = np.asarray(queries, np.float32)
    keys = np.asarray(keys, np.float32)
    values = np.asarray(values, np.float32)
    valid_len = np.asarray(valid_len)
    W_q = np.asarray(W_q, np.float32)
    W_k = np.asarray(W_k, np.float32)
    W_v = np.asarray(W_v, np.float32)
    W_o = np.asarray(W_o, np.float32)

    kcs = [max(1, min(16, -(-int(valid_len[b]) // 128))) for b in range(B)]
    nc = _build(kcs)
    maps = _in_maps(kcs, queries, keys, values, valid_len, W_q, W_k, W_v, W_o)
    res = run_bass_kernel_spmd(nc, maps, list(range(8)), **(_run_kwargs or {}))
    out = np.zeros((B, S, D), np.float32)
    for c in range(8):
        out += res.results[c]["out"].astype(np.float32)
    if _run_kwargs:
        _cache["last_results"] = res
    return out


# revision 18
# speedup vs baseline: 1.0386x; 1.0386x over previous
"""Multi-head attention (B=4, S=2048, D=1024, H=16) on 8 Trainium2 cores.

Sharding (v5): core c -> head-pair p = c (2 heads, 128 output dims), all 4
batches.  This makes the valid_len truncation SPMD-uniform: every core runs
the same per-batch k-loop trip counts kc_b = ceil(valid_len[b]/128), so the
~50% of attention work beyond the key-padding boundary is simply never
emitted.  W_o is row-split by head-pair; each core emits a full-shape
[B, S, D] fp16 partial and the host sums the 8 partials.

Masking: the host zeroes xv columns at k >= valid_len[b] and supplies a
masked ones-column, so masked keys contribute exactly 0 to both the
attention*V accumulation and the softmax denominator.  exp then needs no
bias at all (scores at masked-but-computed boundary columns are finite).

Device layout notes:
  - matmul computes lhsT.T @ rhs with contraction on the partition dim.
  - Q/K are produced transposed ([dout, s]) so scores come out as
    scores^T [k, q]; 1/sqrt(hd) folds into the ScalarE exp op's scale.
  - V is natural [s, dout] with a (host-masked) ones column per head, so the
    attention*V matmul also emits the softmax denominators.
  - All matmul inputs bf16 (4x faster PE than fp32), fp32 PSUM accumulation.
  - PSUM budget: scp 2x2 banks + av 2 + projection 2 = 8; Q/K projection of
    batch b+1 is emitted ahead of attention of batch b so the PE fills the
    gaps of the ACT-bound attention pipeline.
"""

import contextlib

import numpy as np
import ml_dtypes

import concourse.bacc as bacc
import concourse.mybir as mybir
import concourse.tile as tile
from concourse.bass_utils import run_bass_kernel_spmd

BF16 = mybir.dt.bfloat16
F16 = mybir.dt.float16
F32 = mybir.dt.float32
AF = mybir.ActivationFunctionType

B, S, D, H, HD = 4, 2048, 1024, 16, 64
NQB = S // 512        # query blocks of 512

_cache = {}


def _qk_proj_b(nc, b, kcb, ap, stream, psum, wq_sb, wk_sb, qT_sb, kT_sb):
    """Q/K projection for one batch (both heads of the pair = 128 dims)."""
    for qb in range(NQB):
        psq = psum.tile([128, 512], F32, tag="pqk", name="psq")
        for dj in range(8):
            xqt = stream.tile([128, 512], BF16, tag="xq", name="xqt")
            nc.sync.dma_start(
                xqt[:],
                ap[f"xq{b}"][dj * 128:(dj + 1) * 128,
                             qb * 512:(qb + 1) * 512])
            nc.tensor.matmul(psq[:], wq_sb[dj][:], xqt[:],
                             start=(dj == 0), stop=(dj == 7))
        nc.vector.tensor_copy(qT_sb[b][:, qb * 512:(qb + 1) * 512], psq[:])
    nk = kcb * 128
    for kb in range((nk + 511) // 512):
        n = min(512, nk - kb * 512)
        psk = psum.tile([128, 512], F32, tag="pqk", name="psk")
        for dj in range(8):
            xkt = stream.tile([128, 512], BF16, tag="xk", name="xkt")
            nc.sync.dma_start(
                xkt[:, 0:n],
                ap[f"xk{b}"][dj * 128:(dj + 1) * 128,
                             kb * 512:kb * 512 + n])
            nc.tensor.matmul(psk[:, 0:n], wk_sb[dj][:], xkt[:, 0:n],
                             start=(dj == 0), stop=(dj == 7))
        nc.vector.tensor_copy(
            kT_sb[b][:, kb * 512:kb * 512 + n], psk[:, 0:n])


def _v_proj_b(nc, b, kcb, ap, stream, psum, wv_sb, v_sb, vm_sb):
    """V projection for one batch, natural [s, dout] + host-masked ones
    column per head.  v_sb[b][sc] is [128, 2, HD+1]."""
    xv_sb = []
    for dj in range(8):
        xvt = stream.tile([128, 2048], BF16, tag="xv", name="xvt", bufs=9)
        nc.sync.dma_start(
            xvt[:, 0:kcb * 128],
            ap[f"xv{b}"][dj * 128:(dj + 1) * 128, :])
        xv_sb.append(xvt)
    for sc in range(kcb):
        psv = psum.tile([128, 2, HD], F32, tag="pv", name="psv")
        for dj in range(8):
            nc.tensor.matmul(
                psv[:], xv_sb[dj][:, sc * 128:(sc + 1) * 128],
                wv_sb[dj][:], start=(dj == 0), stop=(dj == 7))
        nc.vector.tensor_copy(v_sb[b][sc][:, :, 0:HD], psv[:])
        nc.vector.tensor_copy(
            v_sb[b][sc][:, :, HD], vm_sb[:, b * 16 + sc, :])


def _attention_b(nc, b, kcb, psum, expool, wrk, qT_sb, kT_sb, v_sb, ctx_sb):
    """Attention for one batch (2 heads): ACT-bound pipeline over (qb, kc)."""
    for qb in range(4):
        av = psum.tile([HD + 1, 2, 512], F32, tag="av", name="av")
        for kc in range(kcb):
            scp = psum.tile([128, 2, 512], F32, tag="sc", name="scp", bufs=2)
            for h2 in range(2):
                nc.tensor.matmul(
                    scp[:, h2, :],
                    kT_sb[b][64 * h2:64 * h2 + 64, kc * 128:(kc + 1) * 128],
                    qT_sb[b][64 * h2:64 * h2 + 64, qb * 512:(qb + 1) * 512],
                    start=True, stop=True)
            ex = expool.tile([128, 2, 512], BF16, tag="ex", name="ex")
            nc.scalar.activation(ex[:], scp[:], AF.Exp, scale=0.125)
            for h2 in range(2):
                nc.tensor.matmul(
                    av[:, h2, :], v_sb[b][kc][:, h2, :], ex[:, h2, :],
                    start=(kc == 0), stop=(kc == kcb - 1))
        # Copy av to SBUF (frees the PSUM slot), then normalize off the
        # critical path: ctx[m, q] = av[m, q] / av[64, q].
        avc = wrk.tile([HD + 1, 2, 512], F32, tag="avc", name="avc")
        nc.vector.tensor_copy(avc[:, 0, :], av[:, 0, :])
        nc.scalar.activation(avc[:, 1, :], av[:, 1, :], AF.Copy)
        # Broadcast the sums row to partitions 0..63, then approx-recip there
        # (reciprocal_approx_fast breaks on 1-partition APs, and
        # partition_broadcast only reads partition 0 of its source).
        r0 = wrk.tile([1, 1024], F32, tag="r0", name="r0")
        nc.sync.dma_start(r0[:], avc[HD:HD + 1, :, :])
        bc = wrk.tile([HD, 1024], F32, tag="bc", name="bc")
        nc.gpsimd.partition_broadcast(bc[:], r0[0:1, :])
        recb = wrk.tile([HD, 1024], F32, tag="recb", name="recb")
        nc.vector.reciprocal_approx_fast(recb[:], bc[:])
        nc.vector.tensor_mul(
            ctx_sb[b][0:HD, qb * 512:(qb + 1) * 512],
            avc[0:HD, 0, :], recb[:, 0:512])
        tmp = wrk.tile([HD, 512], BF16, tag="tmpb", name="tmp")
        nc.vector.tensor_mul(tmp[:], avc[0:HD, 1, :], recb[:, 512:1024])
        nc.sync.dma_start(
            ctx_sb[b][HD:128, qb * 512:(qb + 1) * 512], tmp[:])


def _o_proj_b(nc, b, ap, psum, wrk, ctx_sb, wo_sb):
    """Output projection partial for one batch; alternates the two fill-pool
    PSUM slots (pqk/pv) so it pipelines 2-deep."""
    for sc in range(16):
        for ih in range(2):
            tg = "pqk" if (2 * sc + ih) % 2 == 0 else "pv"
            po = psum.tile([128, 512], F32, tag=tg, name="po")
            nc.tensor.matmul(
                po[:], ctx_sb[b][:, sc * 128:(sc + 1) * 128],
                wo_sb[0][:, ih * 512:(ih + 1) * 512],
                start=True, stop=True)
            ot = wrk.tile([128, 512], F16, tag="ot", name="ot", bufs=4)
            if ih == 0:
                nc.vector.tensor_copy(ot[:], po[:])
            else:
                nc.scalar.activation(ot[:], po[:], AF.Copy)
            nc.sync.dma_start(
                ap["out"][b, sc * 128:(sc + 1) * 128,
                          ih * 512:(ih + 1) * 512], ot[:])


def _emit(nc, tc, ap, kcs):
    es = contextlib.ExitStack()
    with es:
        const = es.enter_context(tc.tile_pool(name="const", bufs=1))
        resid = es.enter_context(tc.tile_pool(name="resid", bufs=1))
        stream = es.enter_context(tc.tile_pool(name="stream", bufs=3))
        expool = es.enter_context(tc.tile_pool(name="expool", bufs=3))
        wrk = es.enter_context(tc.tile_pool(name="wrk", bufs=2))

        # constants: per-dj [din-chunk, dout=128] weight tiles for the pair
        wq_sb = [const.tile([128, 128], BF16, tag=f"wq{i}", name=f"wq{i}")
                 for i in range(8)]
        wk_sb = [const.tile([128, 128], BF16, tag=f"wk{i}", name=f"wk{i}")
                 for i in range(8)]
        wv_sb = [const.tile([128, 2, HD], BF16, tag=f"wv{i}", name=f"wv{i}")
                 for i in range(8)]
        wo_sb = [const.tile([128, D], BF16, tag="wo", name="wo")]
        vm_sb = const.tile([128, 64, 2], BF16, tag="vmask", name="vmask")
        nc.sync.dma_start(vm_sb[:], ap["vones"])
        for i in range(8):
            nc.sync.dma_start(wq_sb[i][:], ap["wq"][i * 128:(i + 1) * 128, :])
            nc.sync.dma_start(wk_sb[i][:], ap["wk"][i * 128:(i + 1) * 128, :])
            nc.sync.dma_start(wv_sb[i][:],
                              ap["wv"][i * 128:(i + 1) * 128, :, :])
        nc.sync.dma_start(wo_sb[0][:], ap["wo"])

        # residents (per batch)
        qT_sb = [resid.tile([128, S], BF16, tag=f"qT{b}", name=f"qT{b}")
                 for b in range(B)]
        kT_sb = [resid.tile([128, kcs[b] * 128], BF16, tag=f"kT{b}",
                            name=f"kT{b}") for b in range(B)]
        ctx_sb = [resid.tile([128, S], BF16, tag=f"ctx{b}", name=f"ctx{b}")
                  for b in range(B)]
        v_sb = [[resid.tile([128, 2, HD + 1], BF16, tag=f"v{b}_{i}",
                            name=f"v{b}_{i}") for i in range(kcs[b])]
                for b in range(B)]

        # Fill pool (2 banks: pqk + pv) carries Q/K/V projections and the
        # O-projection; they run in the PE gaps of the ACT-bound attention.
        order = sorted(range(B), key=lambda b: -kcs[b])
        with tc.tile_pool(name="fill_psum", bufs=1, space="PSUM") as fill:
            b0 = order[0]
            _qk_proj_b(nc, b0, kcs[b0], ap, stream, fill,
                       wq_sb, wk_sb, qT_sb, kT_sb)
            _v_proj_b(nc, b0, kcs[b0], ap, stream, fill, wv_sb, v_sb, vm_sb)
            with tc.tile_pool(name="at_psum", bufs=1, space="PSUM") as at_psum:
                # attention first = higher scheduler priority; projections and
                # O fill the PE gaps of the ACT-bound pipeline.
                for i, b in enumerate(order):
                    _attention_b(nc, b, kcs[b], at_psum, expool, wrk,
                                 qT_sb, kT_sb, v_sb, ctx_sb)
                    if i + 1 < B:
                        nb = order[i + 1]
                        _qk_proj_b(nc, nb, kcs[nb], ap, stream, fill,
                                   wq_sb, wk_sb, qT_sb, kT_sb)
                        _v_proj_b(nc, nb, kcs[nb], ap, stream, fill,
                                  wv_sb, v_sb, vm_sb)
                    _o_proj_b(nc, b, ap, fill, wrk, ctx_sb, wo_sb)


def _build(kcs):
    key = ("nc", tuple(kcs))
    if key in _cache:
        return _cache[key]
    nc = bacc.Bacc("TRN2", target_bir_lowering=False, debug=False, num_devices=8)
    ap = {"wq": nc.dram_tensor("wq", [D, 128], BF16, kind="ExternalInput").ap(),
          "wk": nc.dram_tensor("wk", [D, 128], BF16, kind="ExternalInput").ap(),
          "wv": nc.dram_tensor("wv", [D, 2, HD], BF16, kind="ExternalInput").ap(),
          "wo": nc.dram_tensor("wo", [128, D], BF16, kind="ExternalInput").ap(),
          "vones": nc.dram_tensor("vones", [128, 64, 2], BF16,
                                  kind="ExternalInput").ap(),
          "out": nc.dram_tensor("out", [B, S, D], F16,
                                kind="ExternalOutput").ap()}
    for b in range(B):
        ap[f"xq{b}"] = nc.dram_tensor(f"xq{b}", [D, S], BF16,
                                      kind="ExternalInput").ap()
        ap[f"xk{b}"] = nc.dram_tensor(f"xk{b}", [D, kcs[b] * 128], BF16,
                                      kind="ExternalInput").ap()
        ap[f"xv{b}"] = nc.dram_tensor(f"xv{b}", [D, kcs[b] * 128], BF16,
                                      kind="ExternalInput").ap()
    with tile.TileContext(nc) as tc:
        _emit(nc, tc, ap, kcs)
    nc.compile()
    _cache[key] = nc
    return nc


def _in_maps(kcs, queries, keys, values, valid_len, W_q, W_k, W_v, W_o):
    bf = ml_dtypes.bfloat16
    # host-masked ones column: 1 where k < valid_len[b], else 0
    # vones[p, b*16+sc, h] = 1 if sc*128+p < valid_len[b] else 0
    kpos = np.arange(16 * 128).reshape(16, 128)
    vones = np.zeros((128, 64, 2), bf)
    for b in range(B):
        v1 = (kpos < int(valid_len[b])).astype(bf)  # [16, 128]
        vones[:, b * 16:(b + 1) * 16, :] = v1.T[:, :, None]
    maps = []
    for c in range(8):
        j0 = 128 * c
        m = {
            "wq": np.ascontiguousarray(W_q[j0:j0 + 128, :].T).astype(bf),
            "wk": np.ascontiguousarray(W_k[j0:j0 + 128, :].T).astype(bf),
            "wv": np.ascontiguousarray(
                W_v[j0:j0 + 128, :].T).astype(bf).reshape(D, 2, HD),
            "wo": np.ascontiguousarray(W_o[:, j0:j0 + 128].T).astype(bf),
            "vones": vones,
        }
        for b in range(B):
            nk = kcs[b] * 128
            xv = values[b][:nk].T.copy()      # [D, nk]
            xv[:, int(valid_len[b]):] = 0.0   # mask padding rows of v
            m[f"xq{b}"] = np.ascontiguousarray(queries[b].T).astype(bf)
            m[f"xk{b}"] = np.ascontiguousarray(keys[b][:nk].T).astype(bf)
            m[f"xv{b}"] = xv.astype(bf)
        maps.append(m)
    return maps


def kernel(queries, keys, values, valid_len, W_q, W_k, W_v, W_o, _run_kwargs=None):
    queries = np.asarray(queries, np.float32)
    keys = np.asarray(keys, np.float32)
    values = np.asarray(values, np.float32)
    valid_len = np.asarray(valid_len)
    W_q = np.asarray(W_q, np.float32)
    W_k = np.asarray(W_k, np.float32)
    W_v = np.asarray(W_v, np.float32)
    W_o = np.asarray(W_o, np.float32)

    kcs = [max(1, min(16, -(-int(valid_len[b]) // 128))) for b in range(B)]
    nc = _build(kcs)
    maps = _in_maps(kcs, queries, keys, values, valid_len, W_q, W_k, W_v, W_o)
    res = run_bass_kernel_spmd(nc, maps, list(range(8)), **(_run_kwargs or {}))
    out = np.zeros((B, S, D), np.float32)
    for c in range(8):
        out += res.results[c]["out"].astype(np.float32)
    if _run_kwargs:
        _cache["last_results"] = res
    return out
